# revision 10
# baseline (speedup 1.0000x reference)
"""Trainium2 Bass kernel for CoocOpModel.

out[b,s,z] = sum_{i,j} func[b,s,i] * cooc[i,j,z] * arg[b,s,j]
  with func = func_and_arg[..., :128], arg = func_and_arg[..., 128:]

Shapes (hardcoded): func_and_arg [4,1024,256] f32, cooccurrences [128,128,128] f32,
out [4,1024,128] f32.  D = 128, tokens T = 4096.

Strategy: data-parallel over tokens across 8 cores (512 tokens/core).

Per-core math (t = local token index in [0,512)):
  Lane packing: each SBUF partition (matmul contraction lane) is a pair
  lane = (i_sub, j_sub), i_sub in [0,8), j_sub in [0,16).
  Matmul m = (b, c), b in [0,16), c in [0,8) contracts 128 (i,j) pairs:
    i = 8*b + i_sub,  j = 16*c + j_sub
    out[z, t] += sum_lane  c3[lane, m, z] * G_m[lane, t]
    c3[lane, (m, z)] = cooc[8b+i_sub, 16c+j_sub, z]     (host-rearranged)
    G_m[lane, t]     = f[8b+i_sub, t] * a[16c+j_sub, t] (DVE tensor_tensor)
  via replicated operands in SBUF:
    f_rep[lane, (b, t)] = f[8b+i_sub, t]   (2 MB, 16x replication over j_sub)
    a_rep[lane, (c, t)] = a[16c+j_sub, t]  (1 MB,  8x replication over i_sub)
  so total DMA is ~7.25 MB/core instead of 20.4 MB/core for the naive
  1-i-per-matmul layout (which needs f broadcast to all 128 partitions).

Bottleneck model: DVE tensor_tensor builds G (65536 f16 elems/lane at 2x
mode ~= 34 us) and paces the PE (128 matmuls at ~216 ns warm ~= 28 us).
DMA ~7.25 MB ~= 18-20 us hides under the DVE span.  TT chunks are one
b-block (FD=4096) except the first b is split in half so the pipeline
fills early; all DMA dispatches are issued upfront on the two HWDGE
queues ordered so the first TT's inputs land first.
"""

import sys

sys.path.insert(0, "/opt/trn_rl_repo")

import numpy as np
from contextlib import ExitStack

import concourse.bass as bass
import concourse.tile as tile
from concourse import bacc, mybir
from concourse.bass_utils import run_bass_kernel_spmd

F16 = mybir.dt.float16
F32 = mybir.dt.float32
NP_F16 = np.float16

N_CORES = 8
D = 128
T_TOTAL = 4096
T_CORE = T_TOTAL // N_CORES  # 512

P_I = 8    # i_sub values per lane group
P_J = 16   # j_sub values
NB = 16    # b blocks: i = 8b + i_sub
NCC = 8    # c blocks: j = 16c + j_sub

_NC_CACHE = None


def _build():
    nc = bacc.Bacc("TRN2", target_bir_lowering=False, debug=False, num_devices=N_CORES)

    # host-replicated operands (see _prep_in_maps):
    #   f_in[lane, b*512+t] = f[8b+i_sub, t],  a_in[lane, c*512+t] = a[16c+j_sub, t]
    f_in = nc.dram_tensor("f_rep", [D, NB * T_CORE], F16, kind="ExternalInput").ap()
    a_in = nc.dram_tensor("a_rep", [D, NCC * T_CORE], F16, kind="ExternalInput").ap()
    # c3[lane, m*128 + z] = cooc[8b+i_sub, 16c+j_sub, z], lane=(i_sub,j_sub), m=(b,c)
    c3 = nc.dram_tensor("c3", [D, D * D], F16, kind="ExternalInput").ap()
    out_t = nc.dram_tensor("out_t", [D, T_CORE], F16, kind="ExternalOutput").ap()

    with tile.TileContext(nc) as tc:
        with ExitStack() as ctx:
            const_pool = ctx.enter_context(tc.tile_pool(name="const", bufs=1))
            g_pool = ctx.enter_context(tc.tile_pool(name="g", bufs=3))
            out_pool = ctx.enter_context(tc.tile_pool(name="out", bufs=1))
            psum_pool = ctx.enter_context(
                tc.tile_pool(name="psum", bufs=1, space="PSUM")
            )

            a_rep = const_pool.tile([D, NCC * T_CORE], F16, tag="arep")  # [lane,(c,t)]
            f_rep = const_pool.tile([D, NB * T_CORE], F16, tag="frep")   # [lane,(b,t)]
            c_sb = const_pool.tile([D, D * D], F16, tag="c3")            # [lane,(m,z)]

            # ---- all DMA dispatches upfront ------------------------------
            # q0 = sync, q1 = scalar (two HWDGE queues, FIFO each).
            # TT chunk 0 needs f_rep[b=0] + a_rep[c=0..1]; the first MMs
            # need c3[m=0..1].  Put both TT-0 inputs first on q0 (so one
            # slow queue can't stall the fill) and c3 first on q1.
            q0, q1 = nc.sync, nc.scalar

            q0.dma_start(f_rep[:, 0:T_CORE], f_in[:, 0:T_CORE])
            q0.dma_start(a_rep[:, 0 : 2 * T_CORE], a_in[:, 0 : 2 * T_CORE])
            q0.dma_start(a_rep[:, 2 * T_CORE : 4 * T_CORE], a_in[:, 2 * T_CORE : 4 * T_CORE])
            q0.dma_start(a_rep[:, 4 * T_CORE :], a_in[:, 4 * T_CORE :])
            for b0, b1 in ((1, 4), (4, 10), (10, 16)):
                q0.dma_start(
                    f_rep[:, b0 * T_CORE : b1 * T_CORE],
                    f_in[:, b0 * T_CORE : b1 * T_CORE],
                )

            for m0, m1 in ((0, 16), (16, 48), (48, 80), (80, 112), (112, 128)):
                q1.dma_start(c_sb[:, m0 * D : m1 * D], c3[:, m0 * D : m1 * D])

            # ---- compute: TT chunk -> matmuls, accumulate in one PSUM bank
            # chunks: (b, c0, c1); small head chunks (early pipeline fill)
            # and a split tail chunk (short post-last-TT matmul drain).
            chunks = (
                [(0, 0, 2), (0, 2, 4), (0, 4, 8)]
                + [(b, 0, 8) for b in range(1, NB - 1)]
                + [(NB - 1, 0, 4), (NB - 1, 4, 8)]
            )

            ps = psum_pool.tile([D, T_CORE], F32)
            f_ap = f_rep[:]
            for b, c0, c1 in chunks:
                ncol = (c1 - c0) * T_CORE
                gt = g_pool.tile([D, NCC * T_CORE], F16, tag="g")
                # G[lane, (c, t)] = a_rep[lane, (c, t)] * f_rep[lane, (b fixed, t)]
                f_view = bass.AP(
                    f_ap.tensor,
                    f_ap.offset + b * T_CORE,
                    [f_ap.ap[0], [0, c1 - c0], [1, T_CORE]],
                )
                nc.vector.tensor_mul(
                    gt[:, 0:ncol],
                    a_rep[:, c0 * T_CORE : c1 * T_CORE],
                    f_view,
                )
                for c in range(c0, c1):
                    m = b * NCC + c
                    nc.tensor.matmul(
                        ps[:],
                        c_sb[:, m * D : (m + 1) * D],
                        gt[:, (c - c0) * T_CORE : (c - c0 + 1) * T_CORE],
                        start=(m == 0),
                        stop=(m == D - 1),
                    )

            # evacuate PSUM in halves on the (now idle) DVE so the first
            # half's DMA overlaps the second half's copy; f16 output
            # (host converts back; quantization ~3e-4 rel, well in budget)
            o_sb = out_pool.tile([D, T_CORE], F16, tag="o")
            h = T_CORE // 2
            nc.vector.tensor_copy(o_sb[:, 0:h], ps[:, 0:h])
            q1.dma_start(out_t[:, 0:h], o_sb[:, 0:h])
            nc.vector.tensor_copy(o_sb[:, h:], ps[:, h:])
            q0.dma_start(out_t[:, h:], o_sb[:, h:])

    nc.compile()
    return nc


def _get_nc():
    global _NC_CACHE
    if _NC_CACHE is None:
        _NC_CACHE = _build()
    return _NC_CACHE


def _prep_in_maps(func_and_arg, cooccurrences):
    fa = np.asarray(func_and_arg, dtype=np.float32).reshape(T_TOTAL, 2 * D)
    c = np.asarray(cooccurrences, dtype=np.float32)
    # c3[(i_sub, j_sub), (b, c, z)] = cooc[8b+i_sub, 16c+j_sub, z]
    c3 = np.ascontiguousarray(
        c.reshape(NB, P_I, NCC, P_J, D).transpose(1, 3, 0, 2, 4).reshape(D, D * D)
    ).astype(NP_F16)
    in_maps = []
    for core in range(N_CORES):
        s = fa[core * T_CORE : (core + 1) * T_CORE]  # [512, 256]
        f_tc = s[:, :D].T.astype(NP_F16)  # [128 i, 512 t]
        a_tc = s[:, D:].T.astype(NP_F16)  # [128 j, 512 t]
        # f_rep[(i_sub, j_sub), (b, t)] = f[8b+i_sub, t]
        f_rep = np.ascontiguousarray(
            np.broadcast_to(
                f_tc.reshape(NB, P_I, T_CORE).transpose(1, 0, 2)[:, None, :, :],
                (P_I, P_J, NB, T_CORE),
            )
        ).reshape(D, NB * T_CORE)
        # a_rep[(i_sub, j_sub), (c, t)] = a[16c+j_sub, t]
        a_rep = np.ascontiguousarray(
            np.broadcast_to(
                a_tc.reshape(NCC, P_J, T_CORE).transpose(1, 0, 2)[None, :, :, :],
                (P_I, P_J, NCC, T_CORE),
            )
        ).reshape(D, NCC * T_CORE)
        in_maps.append({"f_rep": f_rep, "a_rep": a_rep, "c3": c3})
    return in_maps


def kernel(func_and_arg: np.ndarray, cooccurrences: np.ndarray) -> np.ndarray:
    assert func_and_arg.shape == (4, 1024, 2 * D)
    assert cooccurrences.shape == (D, D, D)

    in_maps = _prep_in_maps(func_and_arg, cooccurrences)
    nc = _get_nc()
    res = run_bass_kernel_spmd(nc, in_maps, core_ids=list(range(N_CORES)))

    # out_t per core: [z=128, t=512] f16 -> [t, z]; concat -> [4096, 128]
    outs = [res.results[c]["out_t"].astype(np.float32).T for c in range(N_CORES)]
    out = np.concatenate(outs, axis=0).reshape(4, 1024, D)
    return np.ascontiguousarray(out)


# revision 11
# speedup vs baseline: 1.1241x; 1.1241x over previous
"""Trainium2 Bass kernel for CoocOpModel.

out[b,s,z] = sum_{i,j} func[b,s,i] * cooc[i,j,z] * arg[b,s,j]
  with func = func_and_arg[..., :128], arg = func_and_arg[..., 128:]

Shapes (hardcoded): func_and_arg [4,1024,256] f32, cooccurrences [128,128,128] f32,
out [4,1024,128] f32.  D = 128, tokens T = 4096.

Strategy: data-parallel over tokens across 8 cores (512 tokens/core).

Per-core math (t = local token index in [0,512)):
  Lane packing: each SBUF partition (matmul contraction lane) is a pair
  lane = (i_sub, j_sub), i_sub in [0,8), j_sub in [0,16).
  Matmul m = (b, c), b in [0,16), c in [0,8) contracts 128 (i,j) pairs:
    i = 8*b + i_sub,  j = 16*c + j_sub
    out[z, t] += sum_lane  c3[lane, m, z] * G_m[lane, t]
    c3[lane, (m, z)] = cooc[8b+i_sub, 16c+j_sub, z]     (host-rearranged)
    G_m[lane, t]     = f[8b+i_sub, t] * a[16c+j_sub, t] (DVE tensor_tensor)
  via replicated operands in SBUF:
    f_rep[lane, (b, t)] = f[8b+i_sub, t]   (2 MB, 16x replication over j_sub)
    a_rep[lane, (c, t)] = a[16c+j_sub, t]  (1 MB,  8x replication over i_sub)
  so total DMA is ~7.25 MB/core instead of 20.4 MB/core for the naive
  1-i-per-matmul layout (which needs f broadcast to all 128 partitions).

Bottleneck model: DVE tensor_tensor builds G (65536 f16 elems/lane at 2x
mode ~= 34 us) and paces the PE (128 matmuls at ~216 ns warm ~= 28 us).
DMA ~7.25 MB ~= 18-20 us hides under the DVE span.  TT chunks are one
b-block (FD=4096) except the first b is split in half so the pipeline
fills early; all DMA dispatches are issued upfront on the two HWDGE
queues ordered so the first TT's inputs land first.
"""

import sys

sys.path.insert(0, "/opt/trn_rl_repo")

import numpy as np
from contextlib import ExitStack

import concourse.bass as bass
import concourse.tile as tile
from concourse import bacc, mybir
from concourse.bass_utils import run_bass_kernel_spmd

F16 = mybir.dt.float16
F32 = mybir.dt.float32
NP_F16 = np.float16

N_CORES = 8
D = 128
T_TOTAL = 4096
T_CORE = T_TOTAL // N_CORES  # 512

P_I = 8    # i_sub values per lane group
P_J = 16   # j_sub values
NB = 16    # b blocks: i = 8b + i_sub
NCC = 8    # c blocks: j = 16c + j_sub

_NC_CACHE = None


def _build():
    nc = bacc.Bacc("TRN2", target_bir_lowering=False, debug=False, num_devices=N_CORES)

    # host-replicated operands (see _prep_in_maps):
    #   f_in[lane, b*512+t] = f[8b+i_sub, t],  a_in[lane, c*512+t] = a[16c+j_sub, t]
    f_in = nc.dram_tensor("f_rep", [D, NB * T_CORE], F16, kind="ExternalInput").ap()
    a_in = nc.dram_tensor("a_rep", [D, NCC * T_CORE], F16, kind="ExternalInput").ap()
    # c3[lane, m*128 + z] = cooc[8b+i_sub, 16c+j_sub, z], lane=(i_sub,j_sub), m=(b,c)
    c3 = nc.dram_tensor("c3", [D, D * D], F16, kind="ExternalInput").ap()
    out_t = nc.dram_tensor("out_t", [D, T_CORE], F16, kind="ExternalOutput").ap()

    with tile.TileContext(nc) as tc:
        with ExitStack() as ctx:
            const_pool = ctx.enter_context(tc.tile_pool(name="const", bufs=1))
            g_pool = ctx.enter_context(tc.tile_pool(name="g", bufs=3))
            out_pool = ctx.enter_context(tc.tile_pool(name="out", bufs=1))
            psum_pool = ctx.enter_context(
                tc.tile_pool(name="psum", bufs=1, space="PSUM")
            )

            a_rep = const_pool.tile([D, NCC * T_CORE], F16, tag="arep")  # [lane,(c,t)]
            f_rep = const_pool.tile([D, NB * T_CORE], F16, tag="frep")   # [lane,(b,t)]
            c_sb = const_pool.tile([D, D * D], F16, tag="c3")            # [lane,(m,z)]

            # ---- all DMA dispatches upfront ------------------------------
            # q0 = sync, q1 = scalar (two HWDGE queues, FIFO each).
            # TT chunk 0 needs f_rep[b=0] + a_rep[c=0..1]; the first MMs
            # need c3[m=0..1].  Put both TT-0 inputs first on q0 (so one
            # slow queue can't stall the fill) and c3 first on q1.
            q0, q1 = nc.sync, nc.scalar

            q0.dma_start(a_rep[:, 0 : 2 * T_CORE], a_in[:, 0 : 2 * T_CORE])
            q0.dma_start(a_rep[:, 2 * T_CORE : 4 * T_CORE], a_in[:, 2 * T_CORE : 4 * T_CORE])
            q0.dma_start(a_rep[:, 4 * T_CORE :], a_in[:, 4 * T_CORE :])
            for b0, b1 in ((1, 4), (4, 10), (10, 16)):
                q0.dma_start(
                    f_rep[:, b0 * T_CORE : b1 * T_CORE],
                    f_in[:, b0 * T_CORE : b1 * T_CORE],
                )

            q1.dma_start(f_rep[:, 0:T_CORE], f_in[:, 0:T_CORE])
            for k in range(8):
                m0 = k * 16
                q1.dma_start(c_sb[:, m0 * D : (m0 + 16) * D], c3[:, m0 * D : (m0 + 16) * D])

            # ---- compute: TT chunk -> matmuls, accumulate in one PSUM bank
            # chunks: (b, c0, c1); small head chunks (early pipeline fill)
            # and a split tail chunk (short post-last-TT matmul drain).
            chunks = (
                [(0, 0, 2), (0, 2, 4), (0, 4, 8)]
                + [(b, 0, 8) for b in range(1, NB - 1)]
                + [(NB - 1, 0, 4), (NB - 1, 4, 8)]
            )

            ps = psum_pool.tile([D, T_CORE], F32)
            f_ap = f_rep[:]
            for b, c0, c1 in chunks:
                ncol = (c1 - c0) * T_CORE
                gt = g_pool.tile([D, NCC * T_CORE], F16, tag="g")
                # G[lane, (c, t)] = a_rep[lane, (c, t)] * f_rep[lane, (b fixed, t)]
                f_view = bass.AP(
                    f_ap.tensor,
                    f_ap.offset + b * T_CORE,
                    [f_ap.ap[0], [0, c1 - c0], [1, T_CORE]],
                )
                nc.vector.tensor_mul(
                    gt[:, 0:ncol],
                    a_rep[:, c0 * T_CORE : c1 * T_CORE],
                    f_view,
                )
                for c in range(c0, c1):
                    m = b * NCC + c
                    nc.tensor.matmul(
                        ps[:],
                        c_sb[:, m * D : (m + 1) * D],
                        gt[:, (c - c0) * T_CORE : (c - c0 + 1) * T_CORE],
                        start=(m == 0),
                        stop=(m == D - 1),
                    )

            # evacuate PSUM in halves on the (now idle) DVE so the first
            # half's DMA overlaps the second half's copy; f16 output
            # (host converts back; quantization ~3e-4 rel, well in budget)
            o_sb = out_pool.tile([D, T_CORE], F16, tag="o")
            h = T_CORE // 2
            nc.vector.tensor_copy(o_sb[:, 0:h], ps[:, 0:h])
            q1.dma_start(out_t[:, 0:h], o_sb[:, 0:h])
            nc.vector.tensor_copy(o_sb[:, h:], ps[:, h:])
            q0.dma_start(out_t[:, h:], o_sb[:, h:])

    nc.compile()
    return nc


def _get_nc():
    global _NC_CACHE
    if _NC_CACHE is None:
        _NC_CACHE = _build()
    return _NC_CACHE


def _prep_in_maps(func_and_arg, cooccurrences):
    fa = np.asarray(func_and_arg, dtype=np.float32).reshape(T_TOTAL, 2 * D)
    c = np.asarray(cooccurrences, dtype=np.float32)
    # c3[(i_sub, j_sub), (b, c, z)] = cooc[8b+i_sub, 16c+j_sub, z]
    c3 = np.ascontiguousarray(
        c.reshape(NB, P_I, NCC, P_J, D).transpose(1, 3, 0, 2, 4).reshape(D, D * D)
    ).astype(NP_F16)
    in_maps = []
    for core in range(N_CORES):
        s = fa[core * T_CORE : (core + 1) * T_CORE]  # [512, 256]
        f_tc = s[:, :D].T.astype(NP_F16)  # [128 i, 512 t]
        a_tc = s[:, D:].T.astype(NP_F16)  # [128 j, 512 t]
        # f_rep[(i_sub, j_sub), (b, t)] = f[8b+i_sub, t]
        f_rep = np.ascontiguousarray(
            np.broadcast_to(
                f_tc.reshape(NB, P_I, T_CORE).transpose(1, 0, 2)[:, None, :, :],
                (P_I, P_J, NB, T_CORE),
            )
        ).reshape(D, NB * T_CORE)
        # a_rep[(i_sub, j_sub), (c, t)] = a[16c+j_sub, t]
        a_rep = np.ascontiguousarray(
            np.broadcast_to(
                a_tc.reshape(NCC, P_J, T_CORE).transpose(1, 0, 2)[None, :, :, :],
                (P_I, P_J, NCC, T_CORE),
            )
        ).reshape(D, NCC * T_CORE)
        in_maps.append({"f_rep": f_rep, "a_rep": a_rep, "c3": c3})
    return in_maps


def kernel(func_and_arg: np.ndarray, cooccurrences: np.ndarray) -> np.ndarray:
    assert func_and_arg.shape == (4, 1024, 2 * D)
    assert cooccurrences.shape == (D, D, D)

    in_maps = _prep_in_maps(func_and_arg, cooccurrences)
    nc = _get_nc()
    res = run_bass_kernel_spmd(nc, in_maps, core_ids=list(range(N_CORES)))

    # out_t per core: [z=128, t=512] f16 -> [t, z]; concat -> [4096, 128]
    outs = [res.results[c]["out_t"].astype(np.float32).T for c in range(N_CORES)]
    out = np.concatenate(outs, axis=0).reshape(4, 1024, D)
    return np.ascontiguousarray(out)
